# revision 16
# baseline (speedup 1.0000x reference)
"""DST-PredRNN (4-layer ST-LSTM with temporal recall attention) on 8 TRN2 cores.

Strategy: data-parallel over batch (B=16 -> 2 per core), zero inter-core
communication.  Each core runs the full 19-step, 4-layer recurrence on its
batch slice.

Per-core layouts:
  - activations: channels on partitions.  64-ch tensors as (64, ...) tiles;
    b-packed tensors (c_state, history) as (128 = [b0|b1] x 64ch, 1024 hw).
  - conv inputs live in zero-padded SBUF buffers (rows, 2, 36, 36) with
    duplicated +1-column-shifted copies on partitions 64:128 so two 5x5 taps
    pack into one K=128 matmul (15 matmuls instead of 25 per 64-ch conv).
  - layer-0 input (16 ch) packs 5 shifted copies -> 5 K=80 matmuls.
  - convs run as f32r matmuls (tf32-ish, full PE rate), PSUM f32 accumulate.
  - recall attention: scores via fused DVE tensor_tensor_reduce against the
    history streamed from DRAM; softmax on-chip; weighted history sum via
    DVE affine_then_add FMA chain (second stream of the history).
  - time loop is a hardware For_i; history slots are masked with -1e9 so the
    body is t-independent.
"""

import sys
import types as _types

import ml_dtypes
import numpy as np

import concourse.bass as bass
import concourse.mybir as mybir
import concourse.tile as tile
from concourse.bass import ds
from concourse.bass_utils import run_bass_kernel_spmd

# ---------------------------------------------------------------- constants
N_CORES = 8
B = 16
NB = B // N_CORES          # batch per core
T_TOTAL = 20
TS = T_TOTAL - 1           # time steps / outputs
T_IN = 10
NL = 4                     # layers
C = 64                     # hidden channels
CIN = 16                   # frame channels
S = 32                     # spatial
P = S * S                  # 1024
PADW = S + 4               # 36
HL = TS                    # history slots read (19)
HCAP = T_TOTAL             # history slots allocated (20)
NEG = -1e9
SCALE = 1.0 / np.sqrt(C * S * S).astype(np.float32)   # 1/256

F32 = mybir.dt.float32
F32R = mybir.dt.float32r
BF16 = mybir.dt.bfloat16
DT_HPAD = BF16           # h/x0 conv-input buffers (bandwidth/SBUF)
DT_MPAD = F32R           # m recurrence + mem stay full f32 storage
AF = mybir.ActivationFunctionType
OP = mybir.AluOpType

NTILES = [(b, y0) for b in range(NB) for y0 in (0, 16)]

# chunk lists: (dy, dx, K)
CHUNKS_PAIR = [(dy, dx, 128 if dx < 4 else 64) for dy in range(5) for dx in (0, 2, 4)]
CHUNKS_X0 = [(dy, 0, 80) for dy in range(5)]
CHUNKS_WO = [(dy, dx, 128) for dy in range(5) for dx in range(5)]

# xs produces 7 gate groups; permute to [i, f, g, o, i', f', g']
XS_PERM = [0, 1, 2, 6, 3, 4, 5]
# M-tiles of the permuted xs: (bank, rows, size)
XS_MT = [(0, (0, 128), 128), (1, (0, 128), 128), (2, (0, 128), 128), (3, (0, 64), 64)]
HS_MT = [(0, (0, 128), 128), (1, (0, 128), 128)]
MS_MT = [(2, (0, 128), 128), (3, (0, 64), 64)]


# ---------------------------------------------------------------- host prep
def _pack_block(w, chunks):
    """w: (OC, IC, 5, 5) -> (rows, len(chunks)*OC) lhsT block array."""
    oc = w.shape[0]
    rows = 80 if chunks is CHUNKS_X0 else 128
    arr = np.zeros((rows, len(chunks) * oc), np.float32)
    for ci, (dy, dx, k) in enumerate(chunks):
        blk = arr[:, ci * oc:(ci + 1) * oc]
        if chunks is CHUNKS_X0:
            for s in range(5):
                blk[16 * s:16 * (s + 1)] = w[:, :, dy, s].T
        elif chunks is CHUNKS_WO:
            blk[0:128] = w[:, :, dy, dx].T
        else:
            blk[0:64] = w[:, :, dy, dx].T
            if k == 128:
                blk[64:128] = w[:, :, dy, dx + 1].T
    return arr


def _mt_split(w, mts):
    """split (OC,...) conv weight into M-tiles -> list of (OC_i,...) arrays"""
    out, off = [], 0
    for _, _, sz in mts:
        out.append(w[off:off + sz])
        off += sz
    return out


def _prep_weights(params, conv_last_w):
    """returns dict name -> np.ndarray of packed per-layer weights"""
    d = {}
    for i, p in enumerate(params):
        wx = np.asarray(p['wx'], np.float32)
        blocks = [wx[g * C:(g + 1) * C] for g in range(7)]
        wxp = np.concatenate([blocks[g] for g in XS_PERM], axis=0)
        chunks = CHUNKS_X0 if i == 0 else CHUNKS_PAIR
        d[f'wx{i}'] = np.concatenate(
            [_pack_block(m, chunks) for m in _mt_split(wxp, XS_MT)],
            axis=1).astype(ml_dtypes.bfloat16)
        d[f'wh{i}'] = np.concatenate(
            [_pack_block(m, CHUNKS_PAIR) for m in
             _mt_split(np.asarray(p['wh'], np.float32), HS_MT)],
            axis=1).astype(ml_dtypes.bfloat16)
        d[f'wm{i}'] = np.concatenate(
            [_pack_block(m, CHUNKS_PAIR) for m in
             _mt_split(np.asarray(p['wm'], np.float32), MS_MT)], axis=1)
        d[f'wo{i}'] = _pack_block(np.asarray(p['wo'], np.float32), CHUNKS_WO)
        d[f'wl{i}'] = np.asarray(p['wlast'], np.float32)[:, :, 0, 0].T.copy()  # (128, 64)
        bx = np.asarray(p['bx'], np.float32)
        bh = np.asarray(p['bh'], np.float32)
        bm = np.asarray(p['bm'], np.float32)
        bo = np.asarray(p['bo'], np.float32)
        bl = np.asarray(p['blast'], np.float32)
        g = lambda v, j: v[j * C:(j + 1) * C]
        cols = [g(bx, 0) + g(bh, 0),            # i
                g(bx, 1) + g(bh, 1) + 1.0,      # f (+forget bias)
                g(bx, 2) + g(bh, 2),            # g
                g(bx, 6) + g(bh, 3) + bo,       # o
                g(bx, 3) + g(bm, 0),            # i'
                g(bx, 4) + g(bm, 1) + 1.0,      # f'
                g(bx, 5) + g(bm, 2),            # g'
                bl]                              # wlast bias
        d[f'bias{i}'] = np.stack(cols, axis=1)   # (64, 8)
    d['head_w'] = np.asarray(conv_last_w, np.float32)[:, :, 0, 0].T.astype(ml_dtypes.bfloat16)  # (64, 16)
    selr = np.zeros((128, 2), np.float32)
    selr[0:64, 0] = float(SCALE)
    selr[64:128, 1] = float(SCALE)
    d['selr'] = selr
    selb = np.zeros((2, 128), np.float32)
    selb[0, 0:64] = 1.0
    selb[1, 64:128] = 1.0
    d['selb'] = selb
    d['zeros_r'] = np.zeros((128, NB, PADW, PADW), np.float32)
    d['zeros_b'] = np.zeros((128, NB, PADW, PADW), ml_dtypes.bfloat16)
    return d


def _prep_percore(frames, mask_true, n_steps):
    """frames (B,T,32,32,16), mask (B,9,32,32,16) -> list of per-core dicts"""
    frames = np.asarray(frames, np.float32)
    mask = np.asarray(mask_true, np.float32)
    maps = []
    lm = np.full((TS, 2, HL), NEG, np.float32)
    for t in range(TS):
        lm[t, :, :t + 1] = 0.0
    lm = lm.reshape(TS * 2, HL).copy()
    for ci in range(N_CORES):
        bs = slice(ci * NB, (ci + 1) * NB)
        fr = frames[bs, :TS]                           # (NB, 19, 32, 32, 16)
        fr = np.ascontiguousarray(fr.transpose(1, 4, 0, 2, 3))  # (19,16,NB,32,32)
        mf = np.ones((TS, CIN, NB, S, S), np.float32)
        mk = mask[bs]                                  # (NB, 9, 32, 32, 16)
        mf[T_IN:] = mk.transpose(1, 4, 0, 2, 3)
        maps.append({
            'frames_l': fr.reshape(TS * CIN, NB, S, S).copy(),
            'm_full': mf.reshape(TS * CIN, NB, S, S).copy(),
            'lmask': lm,
        })
    return maps


# ---------------------------------------------------------------- walrus fix
_SPLIT_CTR = [0]


def _split_multi_waits(nc, max_waits=1):
    """this container's walrus only supports ONE sync-wait per instruction;
    move extra waits onto same-engine NoOps in front."""
    for f in nc.m.functions:
        for bb in f.blocks:
            insts = list(bb.instructions)
            out = []
            for inst in insts:
                si = inst.sync_info
                waits = list(si.on_wait) if (si is not None and si.on_wait) else []
                if len(waits) > max_waits:
                    for w in waits[:-max_waits]:
                        _SPLIT_CTR[0] += 1
                        nop = mybir.InstNoOp(name=f"I-wsplit-{_SPLIT_CTR[0]}")
                        nop.engine = inst.engine
                        nop.sync_info = mybir.SyncInfo(on_wait=[w], on_update=[])
                        out.append(nop)
                    si.on_wait = waits[-max_waits:]
                out.append(inst)
            bb.instructions = out


# ---------------------------------------------------------------- kernel IR
def build_nc(n_steps=TS, split_waits=True):
    nc = bass.Bass("TRN2", target_bir_lowering=False, debug=False,
                   num_devices=N_CORES)

    frames_l = nc.declare_dram_parameter("frames_l", [TS * CIN, NB, S, S], F32, isOutput=False)
    m_full = nc.declare_dram_parameter("m_full", [TS * CIN, NB, S, S], F32, isOutput=False)
    lmask = nc.declare_dram_parameter("lmask", [TS * 2, HL], F32, isOutput=False)
    wx_d, wh_d, wm_d, wo_d, wl_d, bias_d = [], [], [], [], [], []
    for i in range(NL):
        nmt = len(CHUNKS_X0 if i == 0 else CHUNKS_PAIR)
        xrows = 80 if i == 0 else 128
        xcols = nmt * (128 + 128 + 128 + 64)
        wx_d.append(nc.declare_dram_parameter(f"wx{i}", [xrows, xcols], BF16, isOutput=False))
        wh_d.append(nc.declare_dram_parameter(f"wh{i}", [128, 15 * 256], BF16, isOutput=False))
        wm_d.append(nc.declare_dram_parameter(f"wm{i}", [128, 15 * 192], F32R, isOutput=False))
        wo_d.append(nc.declare_dram_parameter(f"wo{i}", [128, 25 * 64], F32R, isOutput=False))
        wl_d.append(nc.declare_dram_parameter(f"wl{i}", [128, 64], F32R, isOutput=False))
        bias_d.append(nc.declare_dram_parameter(f"bias{i}", [64, 8], F32, isOutput=False))
    head_d = nc.declare_dram_parameter("head_w", [64, 16], BF16, isOutput=False)
    selr_d = nc.declare_dram_parameter("selr", [128, 2], F32, isOutput=False)
    selb_d = nc.declare_dram_parameter("selb", [2, 128], F32, isOutput=False)
    zeros_r = nc.declare_dram_parameter("zeros_r", [128, NB, PADW, PADW], F32R, isOutput=False)
    zeros_b = nc.declare_dram_parameter("zeros_b", [128, NB, PADW, PADW], BF16, isOutput=False)
    out_l = nc.declare_dram_parameter("out_l", [TS * CIN, NB, S, S], F32, isOutput=True)

    hist_d = [nc.dram_tensor(f"hist{i}", [HCAP * 128, P], F32) for i in range(NL)]

    with tile.TileContext(nc) as tc:
        with tc.tile_pool(name="perm", bufs=1) as perm, \
             tc.tile_pool(name="wp", bufs=2) as wp, \
             tc.tile_pool(name="stp", bufs=2) as stp, \
             tc.tile_pool(name="gp", bufs=3) as gp, \
             tc.tile_pool(name="tp", bufs=2) as tp, \
             tc.tile_pool(name="sp", bufs=2) as sp, \
             tc.tile_pool(name="spn", bufs=1) as spn, \
             tc.tile_pool(name="pp", bufs=8, space="PSUM") as pp:

            # ---------------- persistent state ----------------
            x0_pad = perm.tile([80, NB, PADW, PADW], DT_HPAD)
            h_pad = [perm.tile([128, NB, PADW, PADW], DT_HPAD, tag=f"hpad{i}", name=f"hpad{i}") for i in range(NL)]
            m_pad = perm.tile([128, NB, PADW, PADW], DT_MPAD)
            mem_pad = perm.tile([128, NB, PADW, PADW], F32R)
            c_state = [perm.tile([128, P], F32, tag=f"cst{i}", name=f"cst{i}") for i in range(NL)]
            x_gen = perm.tile([CIN, NB, S, S], F32)
            wl_sb = [perm.tile([128, 64], F32R, tag=f"wl{i}", name=f"wlsb{i}") for i in range(NL)]
            bias_sb = [perm.tile([64, 8], F32, tag=f"bias{i}", name=f"biassb{i}") for i in range(NL)]
            head_sb = perm.tile([64, 16], BF16)
            selr_sb = perm.tile([128, 2], F32)
            selb_sb = perm.tile([2, 128], F32)
            s_acc = perm.tile([128, HL], F32)         # per-(b,c) score partials
            c_att = perm.tile([128, P], F32)
            c_att_s = perm.tile([64, NB * P], F32)    # base-0 split of c_att

            for i in range(NL):
                nc.sync.dma_start(wl_sb[i][:], wl_d[i][:])
                nc.sync.dma_start(bias_sb[i][:], bias_d[i][:])
            nc.sync.dma_start(head_sb[:], head_d[:])
            nc.sync.dma_start(selr_sb[:], selr_d[:])
            nc.sync.dma_start(selb_sb[:], selb_d[:])

            # zero init (pads via DMA: walrus rejects f32r/bf16 memset)
            nc.sync.dma_start(x0_pad[:], zeros_b[0:80])
            for i in range(NL):
                nc.sync.dma_start(h_pad[i][:], zeros_b[:])
                nc.vector.memset(c_state[i][:], 0.0)
            nc.sync.dma_start(m_pad[:], zeros_r[:])
            nc.sync.dma_start(mem_pad[:], zeros_r[:])
            nc.vector.memset(x_gen[:], 0.0)
            for i in range(NL):
                for e in range(HCAP):
                    nc.sync.dma_start(hist_d[i][e * 128:(e + 1) * 128, :], c_state[0][:])

            def interior(t4, rows, b, y0, shift=0):
                """(rows, 16, 32) interior window of a padded buffer"""
                return t4[rows[0]:rows[1], b, y0 + 2:y0 + 18, 2 - shift:34 - shift]

            def win(t4, rows, b, y0, dy, dx):
                return t4[rows[0]:rows[1], b, y0 + dy:y0 + dy + 16, dx:dx + 32]

            def layer_step(i, t):
                bias = bias_sb[i]
                # ---------- attention: scores over history ----------
                for e in range(HL):
                    st = stp.tile([128, P], F32, tag="stage")
                    nc.sync.dma_start(st[:], hist_d[i][e * 128:(e + 1) * 128, :])
                    nc.vector.tensor_tensor(c_att[:], c_state[i][:], st[:], OP.mult)
                    nc.vector.tensor_reduce(s_acc[:, e:e + 1], c_att[:],
                                            mybir.AxisListType.X, OP.add)
                ps_red = pp.tile([128, 512], F32, tag="pg")
                nc.tensor.matmul(ps_red[0:2, 0:HL], selr_sb[:], s_acc[:, 0:HL],
                                 start=True, stop=True)
                lrow = sp.tile([2, HL], F32, tag="lrow")
                nc.sync.dma_start(lrow[:], lmask[ds(t * 2, 2), :])
                sc = sp.tile([2, HL], F32, tag="sc")
                nc.vector.tensor_tensor(sc[:], ps_red[0:2, 0:HL], lrow[:], OP.add)
                mx = sp.tile([2, 1], F32, tag="mx")
                nc.vector.tensor_reduce(mx[:], sc[:], mybir.AxisListType.X, OP.max)
                negmx = sp.tile([2, 1], F32, tag="negmx")
                nc.vector.tensor_scalar_mul(negmx[:], mx[:], -1.0)
                ex = sp.tile([2, HL], F32, tag="ex")
                sume = sp.tile([2, 1], F32, tag="sume")
                nc.scalar.activation(ex[:], sc[:], AF.Exp, bias=negmx[:, 0:1],
                                     accum_out=sume[:, 0:1])
                rsum = sp.tile([2, 1], F32, tag="rsum")
                nc.vector.reciprocal(rsum[:], sume[:])
                attn = sp.tile([2, HL], F32, tag="attn")
                nc.vector.tensor_scalar(attn[:], ex[:], rsum[:, 0:1], None, OP.mult)
                ps_bc = pp.tile([128, 512], F32, tag="pg")
                nc.tensor.matmul(ps_bc[0:128, 0:HL], selb_sb[:], attn[:],
                                 start=True, stop=True)
                abc = sp.tile([128, HL], F32, tag="abc")
                nc.scalar.copy(abc[:], ps_bc[0:128, 0:HL])
                # ---------- attention: weighted history sum ----------
                for e in range(HL):
                    st = stp.tile([128, P], F32, tag="stage")
                    nc.sync.dma_start(st[:], hist_d[i][e * 128:(e + 1) * 128, :])
                    if e == 0:
                        nc.vector.tensor_scalar(c_att[:], st[:], abc[:, 0:1], None, OP.mult)
                    else:
                        fmt = stp.tile([128, P], F32, tag="fmt", bufs=2)
                        nc.vector.tensor_scalar(fmt[:], st[:], abc[:, e:e + 1], None, OP.mult)
                        nc.vector.tensor_tensor(c_att[:], c_att[:], fmt[:], OP.add)
                for b in range(NB):
                    nc.sync.dma_start(c_att_s[:, b * P:(b + 1) * P],
                                      c_att[b * 64:(b + 1) * 64, :])

                # ---------- gate convs, bank-major ----------
                x_src = x0_pad if i == 0 else h_pad[i - 1]
                x_chunks = CHUNKS_X0 if i == 0 else CHUNKS_PAIR
                xs_off = [0]
                for _, _, sz in XS_MT:
                    xs_off.append(xs_off[-1] + len(x_chunks) * sz)
                hs_off = [0, 15 * 128]
                ms_off = [0, 15 * 128]
                m_in = m_pad
                m_out = m_pad

                # bank -> list of (dram, col_off, chunks, src, osz)
                banks = {
                    0: [(wx_d[i], xs_off[0], x_chunks, x_src, 128),
                        (wh_d[i], hs_off[0], CHUNKS_PAIR, h_pad[i], 128)],
                    1: [(wx_d[i], xs_off[1], x_chunks, x_src, 128),
                        (wh_d[i], hs_off[1], CHUNKS_PAIR, h_pad[i], 128)],
                    2: [(wx_d[i], xs_off[2], x_chunks, x_src, 128),
                        (wm_d[i], ms_off[0], CHUNKS_PAIR, m_in, 128)],
                    3: [(wx_d[i], xs_off[3], x_chunks, x_src, 64),
                        (wm_d[i], ms_off[1], CHUNKS_PAIR, m_in, 64)],
                }
                gt = {}   # gate sbuf tiles per (name, n)
                for bk in range(4):
                    pg = [pp.tile([128, 512], F32, tag="pg", name=f"pg{bk}_{_n}") for _n in range(len(NTILES))]
                    contribs = banks[bk]
                    for wi, (wdram, coff, chunks, src, osz) in enumerate(contribs):
                        ncols = len(chunks) * osz
                        rows = wdram.shape[0]
                        wb = wp.tile([rows, ncols], wdram.dtype, tag="wblk")
                        nc.sync.dma_start(wb[:, :], wdram[:, coff:coff + ncols])
                        for ci, (dy, dx, K) in enumerate(chunks):
                            lhs = wb[0:K, ci * osz:(ci + 1) * osz]
                            for n, (b, y0) in enumerate(NTILES):
                                nc.tensor.matmul(
                                    pg[n][0:osz, :], lhs,
                                    win(src, (0, K), b, y0, dy, dx),
                                    start=(wi == 0 and ci == 0),
                                    stop=(wi == len(contribs) - 1 and ci == len(chunks) - 1))
                    # drain gates
                    for n in range(len(NTILES)):
                        if bk == 0:
                            gt['i', n] = gp.tile([64, 512], F32, tag="gi", name=f"gi{n}")
                            nc.scalar.activation(gt['i', n][:], pg[n][0:64, :], AF.Sigmoid, bias=bias[:, 0:1])
                            gt['f', n] = gp.tile([64, 512], F32, tag="gf", name=f"gf{n}")
                            nc.scalar.activation(gt['f', n][:], pg[n][64:128, :], AF.Sigmoid, bias=bias[:, 1:2])
                        elif bk == 1:
                            gt['g', n] = gp.tile([64, 512], F32, tag="gg", name=f"gg{n}")
                            nc.scalar.activation(gt['g', n][:], pg[n][0:64, :], AF.Tanh, bias=bias[:, 2:3])
                            gt['op', n] = gp.tile([64, 512], F32, tag="gop", name=f"gop{n}", bufs=4)
                            nc.scalar.activation(gt['op', n][:], pg[n][64:128, :], AF.Identity, bias=bias[:, 3:4])
                        elif bk == 2:
                            gt['i2', n] = gp.tile([64, 512], F32, tag="gi2", name=f"gi2_{n}")
                            nc.scalar.activation(gt['i2', n][:], pg[n][0:64, :], AF.Sigmoid, bias=bias[:, 4:5])
                            gt['f2', n] = gp.tile([64, 512], F32, tag="gf2", name=f"gf2_{n}")
                            nc.scalar.activation(gt['f2', n][:], pg[n][64:128, :], AF.Sigmoid, bias=bias[:, 5:6])
                        else:
                            gt['g2', n] = gp.tile([64, 512], F32, tag="gg2", name=f"gg2_{n}")
                            nc.scalar.activation(gt['g2', n][:], pg[n][0:64, :], AF.Tanh, bias=bias[:, 6:7])

                # ---------- c_new / m_new ----------
                for n, (b, y0) in enumerate(NTILES):
                    cw = slice(y0 * S, y0 * S + 512)
                    ta = tp.tile([64, 512], F32, tag="ta")
                    nc.vector.tensor_tensor(ta[:], gt['i', n][:], gt['g', n][:], OP.mult)
                    tb = tp.tile([64, 512], F32, tag="tb")
                    nc.vector.tensor_tensor(tb[:], gt['f', n][:],
                                            c_att_s[:, b * P + y0 * S: b * P + y0 * S + 512], OP.mult)
                    cnw = tp.tile([64, 512], F32, tag="cnw")
                    nc.vector.tensor_tensor(cnw[:], ta[:], tb[:], OP.add)
                    nc.scalar.copy(c_state[i][b * 64:(b + 1) * 64, cw], cnw[:])
                    nc.gpsimd.tensor_copy(interior(mem_pad, (0, 64), b, y0), cnw[:])

                    ta2 = tp.tile([64, 512], F32, tag="ta", name=f"mta{n}")
                    nc.vector.tensor_tensor(ta2[:], gt['i2', n][:], gt['g2', n][:], OP.mult)
                    tb2 = tp.tile([64, 512], F32, tag="tb", name=f"mtb{n}")
                    nc.vector.tensor_tensor(tb2[:], gt['f2', n][:],
                                            interior(m_in, (0, 64), b, y0).bitcast(F32), OP.mult)
                    mnw = tp.tile([64, 512], F32, tag="mnw")
                    nc.vector.tensor_tensor(mnw[:], ta2[:], tb2[:], OP.add)
                    nc.gpsimd.tensor_copy(interior(m_out, (0, 64), b, y0), mnw[:])
                    nc.gpsimd.tensor_copy(interior(m_out, (64, 128), b, y0, shift=1), mnw[:])
                    nc.gpsimd.tensor_copy(interior(mem_pad, (64, 128), b, y0), mnw[:])

                # ---------- output gate + h_new ----------
                pwo = [pp.tile([128, 512], F32, tag="pg", name=f"pwo{_n}") for _n in range(len(NTILES))]
                wb = wp.tile([128, 25 * 64], F32R, tag="wblk")
                nc.sync.dma_start(wb[:, :], wo_d[i][:, :])
                for ci, (dy, dx, K) in enumerate(CHUNKS_WO):
                    lhs = wb[0:128, ci * 64:(ci + 1) * 64]
                    for n, (b, y0) in enumerate(NTILES):
                        nc.tensor.matmul(pwo[n][0:64, :], lhs,
                                         win(mem_pad, (0, 128), b, y0, dy, dx),
                                         start=(ci == 0), stop=(ci == 24))
                for n, (b, y0) in enumerate(NTILES):
                    osum = tp.tile([64, 512], F32, tag="ta", name=f"osum{n}")
                    nc.vector.tensor_tensor(osum[:], pwo[n][0:64, :], gt['op', n][:], OP.add)
                    og = tp.tile([64, 512], F32, tag="tb", name=f"og{n}")
                    nc.scalar.activation(og[:], osum[:], AF.Sigmoid)
                    pwl = pp.tile([128, 512], F32, tag="pg")
                    nc.tensor.matmul(pwl[0:64, :], wl_sb[i][:],
                                     interior(mem_pad, (0, 128), b, y0),
                                     start=True, stop=True)
                    thw = tp.tile([64, 512], F32, tag="thw")
                    nc.scalar.activation(thw[:], pwl[0:64, :], AF.Tanh, bias=bias[:, 7:8])
                    nc.vector.tensor_tensor(interior(h_pad[i], (0, 64), b, y0),
                                            og[:], thw[:], OP.mult)
                    nc.gpsimd.tensor_copy(
                        interior(h_pad[i], (64, 128), b, y0, shift=1),
                        interior(h_pad[i], (0, 64), b, y0))
                    if i == NL - 1:
                        phd = pp.tile([128, 512], F32, tag="pg")
                        nc.tensor.matmul(phd[0:16, :], head_sb[:],
                                         interior(h_pad[i], (0, 64), b, y0),
                                         start=True, stop=True)
                        nc.scalar.copy(x_gen[:, b, y0:y0 + 16, :], phd[0:16, :])
                # append history
                nc.sync.dma_start(hist_d[i][ds((t + 1) * 128, 128), :], c_state[i][:])

            # ---------------- the time loop ----------------
            def body(t):
                fr = spn.tile([CIN, NB, S, S], F32, tag="fr")
                nc.sync.dma_start(fr[:], frames_l[ds(t * CIN, CIN), :, :, :])
                mt = spn.tile([CIN, NB, S, S], F32, tag="mt")
                nc.sync.dma_start(mt[:], m_full[ds(t * CIN, CIN), :, :, :])
                net = spn.tile([CIN, NB, S, S], F32, tag="net")
                nc.vector.tensor_tensor(net[:], fr[:], x_gen[:], OP.subtract)
                nc.vector.tensor_tensor(fr[:], mt[:], net[:], OP.mult)
                nc.vector.tensor_tensor(net[:], fr[:], x_gen[:], OP.add)
                netb = spn.tile([CIN, NB, S, S], BF16, tag="netb")
                nc.scalar.copy(netb[:], net[:])
                for s in range(5):
                    xi0 = max(0, s - 2)
                    po0 = max(0, 2 - s)
                    wdt = 32 - abs(s - 2)
                    for b in range(NB):
                        nc.sync.dma_start(
                            x0_pad[16 * s:16 * (s + 1), b, 2:34, po0:po0 + wdt],
                            netb[:, b, :, xi0:xi0 + wdt])
                for i in range(NL):
                    layer_step(i, t)
                nc.sync.dma_start(out_l[ds(t * CIN, CIN), :, :, :], x_gen[:])

            if n_steps == TS:
                with tc.For_i(0, TS) as t:
                    body(t)
            else:
                for t in range(n_steps):
                    body(t)

    if split_waits:
        _split_multi_waits(nc)
    return nc


# ---------------------------------------------------------------- entry
_CACHE = {}


def _get_nc(n_steps=TS):
    if n_steps not in _CACHE:
        _CACHE[n_steps] = build_nc(n_steps)
    return _CACHE[n_steps]


def kernel(frames, mask_true, params, conv_last_w, n_steps=TS, trace=False):
    nc = _get_nc(n_steps)
    wd = _prep_weights(params, conv_last_w)
    maps = _prep_percore(frames, mask_true, n_steps)
    for m in maps:
        m.update(wd)
    res = run_bass_kernel_spmd(nc, maps, list(range(N_CORES)), trace=trace)
    outs = []
    for ci in range(N_CORES):
        o = res.results[ci]['out_l'].reshape(TS, CIN, NB, S, S)
        outs.append(o.transpose(2, 0, 3, 4, 1))       # (NB, 19, 32, 32, 16)
    full = np.concatenate(outs, axis=0)               # (16, 19, 32, 32, 16)
    if trace:
        kernel.last_exec_time_ns = res.exec_time_ns
    return full


kernel.last_exec_time_ns = None
